# revision 5
# baseline (speedup 1.0000x reference)
"""Trainium2 Bass kernel for CustomPatchEmbedding.

out[b,n,e] = sum_k patch(b,n)[k] * W[e,k] + bias[e], patches are 16x16x3
windows of x at (start_h, start_w)[b,n].

8 NeuronCores, data-parallel over batch (8 images/core).

Host prep:
 - x re-laid into 369 overlapping 16px-wide bands in HWC order, fp16:
   xb[b, sw, h, 48] = x[b, :, h, sw:sw+16] -> the 768 contiguous fp16
   at ((b*369+sw)*384+sh)*48 are exactly patch (sh, sw) in (ph, pw, c)
   element order, so the row-granular indirect DMA lands dense patches.
 - idxT[p, blk]: one int32 element-offset per token, pre-transposed so a
   single DMA loads every block's indices.
 - weights permuted to [k=(ph,pw,c), E] fp16, bias replicated [128, E].

Device per 128-token block (software-pipelined, PE never idles):
 - indirect DMA gather: 128 rows x 768 fp16 = dense patches [tok, k]
 - 6 PE transposes (fp16, 1 cyc/row) -> psum, Act-engine copy -> SBUF
 - 6x2 accumulating fp16 matmuls vs resident weights -> psum [tok, 768]
 - DVE adds bias (f32), writes fp16 out tile -> DMA out; host upcasts.
PE warmup: 20 dummy transposes ramp the p-state (2.4GHz needs 3us of
continuous busy) before real data arrives.
"""
import numpy as np

import concourse.bass as bass
import concourse.bacc as bacc
import concourse.mybir as mybir
import concourse.tile as tile
from concourse.bass_utils import run_bass_kernel_spmd

B, C, H, W = 64, 3, 384, 384
N, E, P = 576, 768, 16
NCORES = 8
BPC = B // NCORES          # 8 images per core
TOK = BPC * N              # 4608 tokens per core
CPP = C * P * P            # 768
KC = CPP // 128            # 6
BLK = 128
NBLK = TOK // BLK          # 36
NB = W - P + 1             # 369 one-px-step bands
BW = P * C                 # 48 fp16 per band row
WARMUP = 20

f32 = mybir.dt.float32
fp16 = mybir.dt.float16
i32 = mybir.dt.int32

_cached = {}


def build_nc(debug=False):
    nc = bacc.Bacc(trn_type="TRN2", debug=debug)
    xb = nc.dram_tensor("xb", [BPC * NB * H, BW], fp16, kind="ExternalInput")
    idx = nc.dram_tensor("idx", [128, NBLK], i32, kind="ExternalInput")
    wk = nc.dram_tensor("wk", [KC, 128, E], fp16, kind="ExternalInput")
    biasr = nc.dram_tensor("biasr", [128, E], f32, kind="ExternalInput")
    ident_d = nc.dram_tensor("ident", [128, 128], fp16, kind="ExternalInput")
    out = nc.dram_tensor("out", [TOK, E], fp16, kind="ExternalOutput")

    with tile.TileContext(nc) as tc:
        with (
            tc.tile_pool(name="const", bufs=1) as cpool,
            tc.tile_pool(name="gather", bufs=6) as gpool,
            tc.tile_pool(name="lhs", bufs=4) as lpool,
            tc.tile_pool(name="outp", bufs=4) as opool,
            tc.tile_pool(name="psA", bufs=2, space="PSUM") as psa_pool,
            tc.tile_pool(name="psB", bufs=2, space="PSUM") as psb_pool,
            tc.tile_pool(name="psT", bufs=2, space="PSUM") as pst_pool,
        ):
            idx_all = cpool.tile([128, NBLK], i32)
            nc.sync.dma_start(idx_all, idx[:, :])
            wk_s = cpool.tile([128, KC * E], fp16)
            bias_s = cpool.tile([128, E], f32)
            ident_s = cpool.tile([128, 128], fp16)
            nc.sync.dma_start(ident_s, ident_d[:, :])

            # dummy PE transposes: ramp the p-state while the pipeline fills
            with tc.tile_pool(name="warm", bufs=2, space="PSUM") as wpool:
                for _ in range(WARMUP):
                    psum_w = wpool.tile([128, 128], fp16, name="psum_w")
                    nc.tensor.transpose(
                        out=psum_w[:, :], in_=ident_s[:], identity=ident_s[:])

            patchesT_q = []

            def stage_front(blk):
                if blk == 0:
                    nc.sync.dma_start(
                        wk_s[:].rearrange("p (k e) -> p k e", k=KC),
                        wk[:, :, :].rearrange("k p e -> p k e"),
                    )
                    nc.sync.dma_start(bias_s, biasr[:, :])
                patches = gpool.tile([128, CPP], fp16, name="patches")
                nc.gpsimd.indirect_dma_start(
                    out=patches[:, :],
                    out_offset=None,
                    in_=xb[:, :],
                    in_offset=bass.IndirectOffsetOnAxis(
                        ap=idx_all[:, blk:blk + 1], axis=1
                    ),
                )
                patchesT = lpool.tile([128, CPP], fp16, name="patchesT")
                psum_t = pst_pool.tile([128, CPP], fp16, name="psum_t")
                for c in range(KC):
                    nc.tensor.transpose(
                        out=psum_t[:, c * 128:(c + 1) * 128],
                        in_=patches[:, c * 128:(c + 1) * 128],
                        identity=ident_s[:],
                    )
                nc.scalar.activation(
                    patchesT[:, :], psum_t[:, :],
                    mybir.ActivationFunctionType.Copy)
                patchesT_q.append(patchesT)

            def stage_back(blk):
                patchesT = patchesT_q.pop(0)
                psum_a = psa_pool.tile([128, 512], f32, name="psum_a")
                psum_b = psb_pool.tile([128, 256], f32, name="psum_b")
                for c in range(KC):
                    lhsT = patchesT[:, c * 128:(c + 1) * 128]
                    nc.tensor.matmul(
                        psum_a[:, :],
                        lhsT,
                        wk_s[:, c * E: c * E + 512],
                        start=(c == 0), stop=(c == KC - 1),
                    )
                    nc.tensor.matmul(
                        psum_b[:, :],
                        lhsT,
                        wk_s[:, c * E + 512:(c + 1) * E],
                        start=(c == 0), stop=(c == KC - 1),
                    )
                out_s = opool.tile([128, E], fp16, name="out_s")
                nc.vector.tensor_add(out_s[:, 0:512], psum_a[:, :],
                                     bias_s[:, 0:512])
                nc.vector.tensor_add(out_s[:, 512:E], psum_b[:, :],
                                     bias_s[:, 512:E])
                nc.sync.dma_start(out[blk * BLK:(blk + 1) * BLK, :], out_s[:])

            for blk in range(NBLK + 1):
                if blk < NBLK:
                    stage_front(blk)
                if blk >= 1:
                    stage_back(blk - 1)
    nc.finalize()
    return nc


def _host_prep(x, proj_w, proj_b):
    x_hwc = np.ascontiguousarray(
        x.transpose(0, 2, 3, 1)).astype(np.float16)    # [B,H,W,C]
    s = x_hwc.strides
    bands = np.lib.stride_tricks.as_strided(
        x_hwc, shape=(B, NB, H, BW), strides=(s[0], s[2], s[1], s[3]))
    xb = np.ascontiguousarray(bands)                   # [B,369,384,48]

    wk_np = np.ascontiguousarray(
        proj_w.transpose(2, 3, 1, 0).reshape(CPP, E)   # (ph,pw,c),E
    ).reshape(KC, 128, E).astype(np.float16)
    bias_rep = np.ascontiguousarray(
        np.broadcast_to(proj_b[None, :].astype(np.float32), (128, E)))
    ident_np = np.eye(128, dtype=np.float16)
    return xb, wk_np, bias_rep, ident_np


def _prep_core_inputs(xb, start_h, start_w, wk_np, bias_rep, ident_np, core):
    b0 = core * BPC
    xc = xb[b0:b0 + BPC].reshape(-1, BW)
    sh = start_h[b0:b0 + BPC].reshape(TOK).astype(np.int64)
    sw = start_w[b0:b0 + BPC].reshape(TOK).astype(np.int64)
    img = np.repeat(np.arange(BPC, dtype=np.int64), N)
    off = ((img * NB + sw) * H + sh) * BW
    idxT = np.ascontiguousarray(
        off.astype(np.int32).reshape(NBLK, 128).T)     # [128, NBLK]
    return {
        "xb": xc,
        "idx": idxT,
        "wk": wk_np,
        "biasr": bias_rep,
        "ident": ident_np,
    }


def kernel(x, start_h, start_w, proj_w, proj_b, _run_kwargs=None, _return_res=False):
    x = np.asarray(x, dtype=np.float32)
    start_h = np.asarray(start_h, dtype=np.int32)
    start_w = np.asarray(start_w, dtype=np.int32)
    proj_w = np.asarray(proj_w, dtype=np.float32)
    proj_b = np.asarray(proj_b, dtype=np.float32)

    xb, wk_np, bias_rep, ident_np = _host_prep(x, proj_w, proj_b)

    if "nc" not in _cached:
        _cached["nc"] = build_nc()
    nc = _cached["nc"]

    in_maps = [
        _prep_core_inputs(xb, start_h, start_w, wk_np, bias_rep, ident_np, c)
        for c in range(NCORES)
    ]
    res = run_bass_kernel_spmd(
        nc, in_maps, core_ids=list(range(NCORES)), **(_run_kwargs or {})
    )
    out = np.concatenate(
        [r["out"].reshape(BPC, N, E).astype(np.float32) for r in res.results],
        axis=0,
    )
    if _return_res:
        return out, res
    return out
